# revision 11
# baseline (speedup 1.0000x reference)
"""Conv2d(32->32, 3x3, stride 1, pad 1) on X[32,32,224,224] fp32, data-parallel
over 8 NeuronCores (4 images per core).

Per-core algorithm ("full-K quad")
----------------------------------
The conv is computed as PE matmuls with M = 128 = (ho in 0..3 output rows) x
(k = 32 output channels), N = 448 = (u in 0..1 row-quads) x (w in 0..223), in
fp16.  Each output quad starts at h0 = 0 mod 4, so its 6 input-row taps
j = h0 + d (d in 0..5) always split cleanly across the row-rotated SBUF
layout Xr[32*q + c, jd, w] (padded row j = 4*jd + q): d in 0..3 live at
jd = h0/4 (all 128 partitions), d in 4..5 at jd = h0/4 + 1 (partitions
0..63).  One PSUM accumulation group is therefore 6 matmuls: per column
shift s, a K=128 matmul (taps 0..3, weights Wt1) plus a K=64 matmul
(taps 4..5, weights Wt2).  No second rotated X copy is needed.

Matmuls are issued in K-uniform sweeps (all K=128 for a 7-group block,
then all K=64): a K change between consecutive matmuls costs ~126 ns of
PE pipeline drain (measured).  ~24 dummy matmuls are issued before the
first X tile lands so the PE HAM clock-gate is already at 2.4 GHz when
real work starts.

lhsT1[32*d+c, s][32*ho+k] = W[k, c, d-ho, s]   (zero outside 0<=r<3)
lhsT2[32*q+c, s][32*ho+k] = W[k, c, 4+q-ho, s] (zero outside 0<=r<3)

Bias is fused into the PSUM->SBUF eviction (ScalarE/VectorE alternating),
which also narrows to fp16 (output max ~90, fp16 rel step 4.9e-4 -- far
inside the 2e-2 gate; the host widens back to fp32).  Eviction is
partition-preserving: PSUM partition 32*ho+k lands at staging partition
32*G+k with G = h%4 = ho.  Y is stored to DRAM in [n, G, k, h//4, w]
layout: each store descriptor is a contiguous 14*224*2 = 6.3 KB run, and
the (G, k) dims merge to a flat 128 so the runtime splits the store
across all 16 SDMA engines (a non-mergeable outer dim gets only 4 -- the
original [k, h, w] layout's 896 B descriptors ran at ~140 GB/s).  The
host un-permutes with one numpy transpose at the end.  Work is H-sliced
(112 output rows, staged as two 56-row half tiles so stores start early)
for SBUF fit and load/compute overlap.
"""

import sys

import numpy as np

try:
    import concourse.bass as bass  # noqa: F401
except ImportError:  # pragma: no cover
    sys.path.insert(0, "/opt/trn_rl_repo")

import ml_dtypes
import concourse.mybir as mybir
import concourse.tile as tile
from concourse import bacc
from concourse.bass_utils import run_bass_kernel_spmd

NCORES = 8
NB = 4  # images per core
C = 32
K = 32
H = 224
W = 224
WP = 226  # padded width
NQ = 57  # row-quads in the host-rotated layout (228 padded rows / 4)
RS = 112  # output rows per slice
NSLICE = H // RS
NJD = RS // 4 + 1  # row-quads per slice tile (halo included)
NM = RS // 4  # stored row-quads per slice
F32 = mybir.dt.float32
F16 = mybir.dt.float16
AF = mybir.ActivationFunctionType
_NP16 = np.float16


def conv_body(tc, X, Wt1, Wt2, Bias, Y, Warm):
    nc = tc.nc
    with (
        tc.tile_pool(name="const", bufs=1) as cpool,
        tc.tile_pool(name="xpool", bufs=4) as xpool,
        tc.tile_pool(name="ypool", bufs=6) as ypool,
        tc.tile_pool(name="ppool", bufs=8, space="PSUM") as ppool,
    ):
        # w1 leads the sync ring (the PE warm-up needs it immediately);
        # w2/bias ride the scalar ring, which spins up while the first X
        # half-tile is in flight
        w1_sb = cpool.tile([128, 3, 128], F16)
        nc.sync.dma_start(out=w1_sb[:], in_=Wt1)
        w2_sb = cpool.tile([64, 3, 128], F16)
        nc.scalar.dma_start(out=w2_sb[:], in_=Wt2)
        b_sb = cpool.tile([128, 1], F32)
        nc.scalar.dma_start(out=b_sb[:], in_=Bias)

        # PE warm-up: ~24 garbage matmuls on the weight tile while the
        # first X tile is still in flight (HAM un-throttles after ~3.4 us
        # of activity; these run in the DMA shadow).
        wpt = ppool.tile([128, 2, 224], F32, name="pt", tag="pt")
        for d in range(28):
            nc.tensor.matmul(
                wpt[:, :, 0:128],
                w1_sb[:, 0, :],
                w1_sb[:, 0:2, :],
                start=(d == 0),
                stop=(d == 27),
            )
        wsb = ypool.tile([128, 8], F32, name="wsb", tag="wsb")
        nc.scalar.activation(wsb[:, :], wpt[:, 0, 0:8], AF.Identity)
        nc.scalar.dma_start(out=Warm, in_=wsb[:, :])

        NH = NM // 2 + 1  # quads per half-slice tile (halo included)
        for n in range(NB):
            for t in range(NSLICE):
                # two 56-row half-slices, each with its own X half-tile,
                # staging tile and store, so the pipeline advances in
                # ~0.9 MB / 1.6 MB steps
                for blk in range(2):
                    j0 = NM * t + 14 * blk
                    xr = xpool.tile([128, NH, WP], F16, name="xr", tag="xr")
                    nc.sync.dma_start(
                        out=xr[:, :, :], in_=X[n, :, :, j0 : j0 + NH, :]
                    )
                    gs = range(7)
                    ysb = ypool.tile([128, NM // 2, 224], F16, name="ysb", tag="ysb")
                    pts = {
                        i: ppool.tile([128, 2, 224], F32, name="pt", tag="pt")
                        for i in gs
                    }
                    # s-outer sweeps: the stationary operand changes once
                    # per 7 matmuls, not per matmul
                    for s in range(3):
                        for i in gs:
                            nc.tensor.matmul(
                                pts[i][:, :, :],
                                w1_sb[:, s, :],
                                xr[:, 2 * i : 2 * i + 2, s : s + 224],
                                start=(s == 0),
                                stop=False,
                            )
                    for s in range(3):
                        for i in gs:
                            nc.tensor.matmul(
                                pts[i][:, :, :],
                                w2_sb[:, s, :],
                                xr[0:64, 2 * i + 1 : 2 * i + 3, s : s + 224],
                                start=False,
                                stop=(s == 2),
                            )
                            if s == 2:
                                dst = ysb[:, 2 * i : 2 * i + 2, :]
                                if i % 2 == 0:
                                    nc.scalar.activation(
                                        dst,
                                        pts[i][:, :, :],
                                        AF.Identity,
                                        bias=b_sb[:, :],
                                    )
                                else:
                                    nc.vector.tensor_scalar_add(
                                        dst, pts[i][:, :, :], b_sb[:, :]
                                    )
                    nc.scalar.dma_start(
                        out=Y[n, :, :, j0 : j0 + 14, :], in_=ysb[:, :, :]
                    )


def build_nc(nb=NB):
    assert nb == NB
    nc = bacc.Bacc("TRN2", target_bir_lowering=False, debug=False)
    X = nc.dram_tensor("X", [NB, 4, C, NQ, WP], F16, kind="ExternalInput").ap()
    Wt1 = nc.dram_tensor("Wt1", [128, 3, 128], F16, kind="ExternalInput").ap()
    Wt2 = nc.dram_tensor("Wt2", [64, 3, 128], F16, kind="ExternalInput").ap()
    Bias = nc.dram_tensor("bias", [128, 1], F32, kind="ExternalInput").ap()
    Y = nc.dram_tensor("Y", [NB, 4, K, H // 4, 224], F16, kind="ExternalOutput").ap()
    Warm = nc.dram_tensor("warm", [128, 8], F32, kind="ExternalOutput").ap()
    with tile.TileContext(nc) as tc:
        conv_body(tc, X, Wt1, Wt2, Bias, Y, Warm)
    nc.compile()
    return nc


def prep_weights(Wf, b):
    """lhsT1[32d+c, s, 32ho+k] = W[k,c,d-ho,s]; lhsT2[32q+c, s, 32ho+k] =
    W[k,c,4+q-ho,s] (zero outside 0<=r<3)."""
    Wf = np.asarray(Wf, np.float32)
    Wt1 = np.zeros((128, 3, 128), np.float32)
    Wt2 = np.zeros((64, 3, 128), np.float32)
    for d in range(4):
        for ho in range(4):
            r = d - ho
            if 0 <= r <= 2:
                Wt1[32 * d : 32 * (d + 1), :, 32 * ho : 32 * (ho + 1)] = Wf[
                    :, :, r, :
                ].transpose(1, 2, 0)
    for q in range(2):
        for ho in range(4):
            r = 4 + q - ho
            if 0 <= r <= 2:
                Wt2[32 * q : 32 * (q + 1), :, 32 * ho : 32 * (ho + 1)] = Wf[
                    :, :, r, :
                ].transpose(1, 2, 0)
    bias = np.tile(np.asarray(b, np.float32), 4).reshape(128, 1)
    return Wt1.astype(_NP16), Wt2.astype(_NP16), bias


def pad_input(X):
    """Pad to 228x226 and pre-rotate rows: out[n, q, c, jd, w] = row 4*jd + q."""
    X = np.ascontiguousarray(X, np.float32)
    Xp = np.zeros((X.shape[0], C, H + 4, WP), _NP16)
    Xp[:, :, 1 : H + 1, 1 : W + 1] = X
    Xr = Xp.reshape(X.shape[0], C, NQ, 4, WP).transpose(0, 3, 1, 2, 4)
    return np.ascontiguousarray(Xr)


_NC = None


def _get_nc():
    global _NC
    if _NC is None:
        _NC = build_nc(NB)
    return _NC


def kernel(X, W, b, _trace=False):
    Xp = pad_input(X)
    Wt1, Wt2, bias = prep_weights(W, b)
    nc = _get_nc()
    in_maps = [
        {"X": Xp[NB * c : NB * (c + 1)], "Wt1": Wt1, "Wt2": Wt2, "bias": bias}
        for c in range(NCORES)
    ]
    res = run_bass_kernel_spmd(nc, in_maps, list(range(NCORES)), trace=_trace)
    # per-core result: [n, G, k, m, w] fp16 with output row h = 4*m + G
    y2 = np.concatenate([res.results[c]["Y"] for c in range(NCORES)], axis=0)
    out = np.ascontiguousarray(
        y2.transpose(0, 2, 3, 1, 4).astype(np.float32)
    ).reshape(NCORES * NB, 32, 224, 224)
    if _trace:
        return out, res
    return out


# revision 12
# speedup vs baseline: 1.0539x; 1.0539x over previous
"""Conv2d(32->32, 3x3, stride 1, pad 1) on X[32,32,224,224] fp32, data-parallel
over 8 NeuronCores (4 images per core).

Per-core algorithm ("full-K quad")
----------------------------------
The conv is computed as PE matmuls with M = 128 = (ho in 0..3 output rows) x
(k = 32 output channels), N = 448 = (u in 0..1 row-quads) x (w in 0..223), in
fp16.  Each output quad starts at h0 = 0 mod 4, so its 6 input-row taps
j = h0 + d (d in 0..5) always split cleanly across the row-rotated SBUF
layout Xr[32*q + c, jd, w] (padded row j = 4*jd + q): d in 0..3 live at
jd = h0/4 (all 128 partitions), d in 4..5 at jd = h0/4 + 1 (partitions
0..63).  One PSUM accumulation group is therefore 6 matmuls: per column
shift s, a K=128 matmul (taps 0..3, weights Wt1) plus a K=64 matmul
(taps 4..5, weights Wt2).  No second rotated X copy is needed.

Matmuls are issued in K-uniform sweeps (all K=128 for a 7-group block,
then all K=64): a K change between consecutive matmuls costs ~126 ns of
PE pipeline drain (measured).  ~24 dummy matmuls are issued before the
first X tile lands so the PE HAM clock-gate is already at 2.4 GHz when
real work starts.

lhsT1[32*d+c, s][32*ho+k] = W[k, c, d-ho, s]   (zero outside 0<=r<3)
lhsT2[32*q+c, s][32*ho+k] = W[k, c, 4+q-ho, s] (zero outside 0<=r<3)

Bias is fused into the PSUM->SBUF eviction (ScalarE/VectorE alternating),
which also narrows to fp16 (output max ~90, fp16 rel step 4.9e-4 -- far
inside the 2e-2 gate; the host widens back to fp32).  Eviction is
partition-preserving: PSUM partition 32*ho+k lands at staging partition
32*G+k with G = h%4 = ho.  Y is stored to DRAM in [n, G, k, h//4, w]
layout: each store descriptor is a contiguous 14*224*2 = 6.3 KB run, and
the (G, k) dims merge to a flat 128 so the runtime splits the store
across all 16 SDMA engines (a non-mergeable outer dim gets only 4 -- the
original [k, h, w] layout's 896 B descriptors ran at ~140 GB/s).  The
host un-permutes with one numpy transpose at the end.  Work is H-sliced
(112 output rows, staged as two 56-row half tiles so stores start early)
for SBUF fit and load/compute overlap.
"""

import sys

import numpy as np

try:
    import concourse.bass as bass  # noqa: F401
except ImportError:  # pragma: no cover
    sys.path.insert(0, "/opt/trn_rl_repo")

import ml_dtypes
import concourse.mybir as mybir
import concourse.tile as tile
from concourse import bacc
from concourse.bass_utils import run_bass_kernel_spmd

NCORES = 8
NB = 4  # images per core
C = 32
K = 32
H = 224
W = 224
WP = 226  # padded width
NQ = 57  # row-quads in the host-rotated layout (228 padded rows / 4)
RS = 112  # output rows per slice
NSLICE = H // RS
NJD = RS // 4 + 1  # row-quads per slice tile (halo included)
NM = RS // 4  # stored row-quads per slice
F32 = mybir.dt.float32
F16 = mybir.dt.float16
AF = mybir.ActivationFunctionType
_NP16 = np.float16


def conv_body(tc, X, Wt1, Wt2, Bias, Y, Warm):
    nc = tc.nc
    with (
        tc.tile_pool(name="const", bufs=1) as cpool,
        tc.tile_pool(name="xpool", bufs=4) as xpool,
        tc.tile_pool(name="ypool", bufs=6) as ypool,
        tc.tile_pool(name="ppool", bufs=8, space="PSUM") as ppool,
    ):
        # w1 leads the sync ring (the PE warm-up needs it immediately);
        # w2/bias ride the scalar ring, which spins up while the first X
        # half-tile is in flight
        w1_sb = cpool.tile([128, 3, 128], F16)
        nc.sync.dma_start(out=w1_sb[:], in_=Wt1)
        w2_sb = cpool.tile([64, 3, 128], F16)
        nc.scalar.dma_start(out=w2_sb[:], in_=Wt2)
        b_sb = cpool.tile([128, 1], F32)
        nc.scalar.dma_start(out=b_sb[:], in_=Bias)

        # PE warm-up: ~24 garbage matmuls on the weight tile while the
        # first X tile is still in flight (HAM un-throttles after ~3.4 us
        # of activity; these run in the DMA shadow).
        wpt = ppool.tile([128, 2, 224], F32, name="pt", tag="pt")
        for d in range(28):
            nc.tensor.matmul(
                wpt[:, :, 0:128],
                w1_sb[:, 0, :],
                w1_sb[:, 0:2, :],
                start=(d == 0),
                stop=(d == 27),
            )
        wsb = ypool.tile([128, 8], F32, name="wsb", tag="wsb")
        nc.scalar.activation(wsb[:, :], wpt[:, 0, 0:8], AF.Identity)
        nc.scalar.dma_start(out=Warm, in_=wsb[:, :])

        NH = NM // 2 + 1  # quads per half-slice tile (halo included)
        for n in range(NB):
            for t in range(NSLICE):
                # two 56-row half-slices, each with its own X half-tile,
                # staging tile and store, so the pipeline advances in
                # ~0.9 MB / 1.6 MB steps
                for blk in range(2):
                    j0 = NM * t + 14 * blk
                    xr = xpool.tile([128, NH, WP], F16, name="xr", tag="xr")
                    nc.sync.dma_start(
                        out=xr[:, :, :], in_=X[n, :, :, j0 : j0 + NH, :]
                    )
                    gs = range(7)
                    ysb = ypool.tile([128, NM // 2, 224], F16, name="ysb", tag="ysb")
                    pts = {
                        i: ppool.tile([128, 2, 224], F32, name="pt", tag="pt")
                        for i in gs
                    }
                    for i in gs:
                        for s in range(3):
                            nc.tensor.matmul(
                                pts[i][:, :, :],
                                w1_sb[:, s, :],
                                xr[:, 2 * i : 2 * i + 2, s : s + 224],
                                start=(s == 0),
                                stop=False,
                            )
                    for i in gs:
                        for s in range(3):
                            nc.tensor.matmul(
                                pts[i][:, :, :],
                                w2_sb[:, s, :],
                                xr[0:64, 2 * i + 1 : 2 * i + 3, s : s + 224],
                                start=False,
                                stop=(s == 2),
                            )
                        dst = ysb[:, 2 * i : 2 * i + 2, :]
                        if i % 2 == 0:
                            nc.scalar.activation(
                                dst, pts[i][:, :, :], AF.Identity, bias=b_sb[:, :]
                            )
                        else:
                            nc.vector.tensor_scalar_add(
                                dst, pts[i][:, :, :], b_sb[:, :]
                            )
                    nc.scalar.dma_start(
                        out=Y[n, :, :, j0 : j0 + 14, :], in_=ysb[:, :, :]
                    )


def build_nc(nb=NB):
    assert nb == NB
    nc = bacc.Bacc("TRN2", target_bir_lowering=False, debug=False)
    X = nc.dram_tensor("X", [NB, 4, C, NQ, WP], F16, kind="ExternalInput").ap()
    Wt1 = nc.dram_tensor("Wt1", [128, 3, 128], F16, kind="ExternalInput").ap()
    Wt2 = nc.dram_tensor("Wt2", [64, 3, 128], F16, kind="ExternalInput").ap()
    Bias = nc.dram_tensor("bias", [128, 1], F32, kind="ExternalInput").ap()
    Y = nc.dram_tensor("Y", [NB, 4, K, H // 4, 224], F16, kind="ExternalOutput").ap()
    Warm = nc.dram_tensor("warm", [128, 8], F32, kind="ExternalOutput").ap()
    with tile.TileContext(nc) as tc:
        conv_body(tc, X, Wt1, Wt2, Bias, Y, Warm)
    nc.compile()
    return nc


def prep_weights(Wf, b):
    """lhsT1[32d+c, s, 32ho+k] = W[k,c,d-ho,s]; lhsT2[32q+c, s, 32ho+k] =
    W[k,c,4+q-ho,s] (zero outside 0<=r<3)."""
    Wf = np.asarray(Wf, np.float32)
    Wt1 = np.zeros((128, 3, 128), np.float32)
    Wt2 = np.zeros((64, 3, 128), np.float32)
    for d in range(4):
        for ho in range(4):
            r = d - ho
            if 0 <= r <= 2:
                Wt1[32 * d : 32 * (d + 1), :, 32 * ho : 32 * (ho + 1)] = Wf[
                    :, :, r, :
                ].transpose(1, 2, 0)
    for q in range(2):
        for ho in range(4):
            r = 4 + q - ho
            if 0 <= r <= 2:
                Wt2[32 * q : 32 * (q + 1), :, 32 * ho : 32 * (ho + 1)] = Wf[
                    :, :, r, :
                ].transpose(1, 2, 0)
    bias = np.tile(np.asarray(b, np.float32), 4).reshape(128, 1)
    return Wt1.astype(_NP16), Wt2.astype(_NP16), bias


def pad_input(X):
    """Pad to 228x226 and pre-rotate rows: out[n, q, c, jd, w] = row 4*jd + q."""
    X = np.ascontiguousarray(X, np.float32)
    Xp = np.zeros((X.shape[0], C, H + 4, WP), _NP16)
    Xp[:, :, 1 : H + 1, 1 : W + 1] = X
    Xr = Xp.reshape(X.shape[0], C, NQ, 4, WP).transpose(0, 3, 1, 2, 4)
    return np.ascontiguousarray(Xr)


_NC = None


def _get_nc():
    global _NC
    if _NC is None:
        _NC = build_nc(NB)
    return _NC


def kernel(X, W, b, _trace=False):
    Xp = pad_input(X)
    Wt1, Wt2, bias = prep_weights(W, b)
    nc = _get_nc()
    in_maps = [
        {"X": Xp[NB * c : NB * (c + 1)], "Wt1": Wt1, "Wt2": Wt2, "bias": bias}
        for c in range(NCORES)
    ]
    res = run_bass_kernel_spmd(nc, in_maps, list(range(NCORES)), trace=_trace)
    # per-core result: [n, G, k, m, w] fp16 with output row h = 4*m + G
    y2 = np.concatenate([res.results[c]["Y"] for c in range(NCORES)], axis=0)
    out = np.ascontiguousarray(
        y2.transpose(0, 2, 3, 1, 4).astype(np.float32)
    ).reshape(NCORES * NB, 32, 224, 224)
    if _trace:
        return out, res
    return out
